# revision 1
# baseline (speedup 1.0000x reference)
"""Trainium2 Bass kernel for nn_FComb_79319456023150 (dense_cnn).

Per-pixel MLP over a 96^3 volume: four 1x1x1 convs (38->32->32->32->1 channels
with relu between). z is batch-constant, so w1[:, 32:38] @ z folds into the
layer-1 bias and every layer becomes a K=32 channel GEMM.

Sharding: spatial (outermost X axis) across 8 cores, 110592 pixels each.
Weights/biases replicated.

Device layout per core: the host restripes each shard to [128, 27648] = 4
pixel-blocks x 32 channels on partitions, pixels on the free dim. Each layer
is computed with a BLOCK-DIAGONAL [128, 128] weight (4 copies of W^T on the
diagonal), so one full-array float32r matmul per 512-pixel chunk applies the
32x32 GEMM to all 4 pixel blocks at once (1 col/cycle). The final layer
(wl: 1x32) uses one sparse [128, 128] weight per chunk whose outputs land on
contiguous partitions 4c+m; accumulating the chunk matmuls into one PSUM
bank packs a whole super-chunk's output into rows 0..OROWS-1 for a single
cheap evacuation op and batched, affine output DMAs.

Relu+bias rides the mandatory PSUM->SBUF crossing as ONE whole-crossing op
per layer, alternating between ScalarE (activation Relu w/ bias) and VectorE
(fused tensor_scalar add+max) by (s+layer) parity — these two engines are
the throughput bound (fp32-from-PSUM is 1x on both), and whole ops amortize
their fixed per-op cost best while keeping the two engines' dependency
chains decoupled. FOUR independent super-chunk pipelines (s%4), each owning
one 2-bank PSUM slot (the L4 accumulator reuses the slot after relu3 drains
it), keep both engines ~75% busy. Input DMAs ramp up (4 single-sc loads,
then 3-sc batches) so the pipeline starts early; each HWDGE dma_start costs
~0.65us of issuing-sequencer time, hence the batching.
"""

import sys

import numpy as np

if "/opt/trn_rl_repo" not in sys.path:
    sys.path.insert(0, "/opt/trn_rl_repo")

C = 32          # channels per layer
P = 128         # SBUF/PSUM partitions
RG = 4          # pixel blocks stacked on the partition dim (128/32)
NCHUNK = 2      # 512-wide chunks per super-chunk (PSUM big tile = 2 banks)
CH = 512        # chunk width (one PSUM bank of fp32)
SCW = NCHUNK * CH                    # 1536 free-dim columns per super-chunk
VOL = 96 * 96 * 96                   # full volume
NCORES = 8
NPIX = VOL // NCORES                 # 110592 pixels per core
FREE = NPIX // RG                    # 27648 free-dim columns per core
NSC = FREE // SCW                    # 18 super-chunks per core
OROWS = RG * NCHUNK                  # 12 packed output rows per super-chunk
assert FREE % SCW == 0



def _pick_group(nsc, target):
    for g in range(min(target, nsc), 0, -1):
        if nsc % g == 0:
            return g
    return 1


def _build_nc(npix=NPIX, use_f32r=True, stagger=False, mirror=False):
    import concourse.mybir as mybir
    from concourse import bacc
    from concourse.tile import TileContext
    from concourse.tile_rust import add_dep_helper

    f32 = mybir.dt.float32
    f32r = mybir.dt.float32r if use_f32r else mybir.dt.float32
    Alu = mybir.AluOpType
    Act = mybir.ActivationFunctionType

    free = npix // RG
    nsc = free // SCW
    assert free % SCW == 0 and nsc >= 1
    gin = _pick_group(nsc, 3)        # super-chunks per input DMA
    gout = _pick_group(nsc, 27)       # super-chunks per output tile/DMA group

    nc = bacc.Bacc()
    fm = nc.dram_tensor("fm", [P, free], f32r, kind="ExternalInput")
    wst = nc.dram_tensor("wst", [P, (3 + NCHUNK) * P], f32r, kind="ExternalInput")
    bias = nc.dram_tensor("bias", [P, 4], f32, kind="ExternalInput")
    out = nc.dram_tensor("out", [npix], f32, kind="ExternalOutput")

    # out[m*free + s*SCW + c*CH + n] viewed for batched affine stores
    out_r = out.rearrange(
        "(m go g c n) -> m go g c n", m=RG, go=nsc // gout, g=gout, c=NCHUNK, n=CH
    )

    with TileContext(nc) as tc:
        with (
            tc.tile_pool(name="const", bufs=1) as constp,
            tc.tile_pool(name="data", bufs=4) as datap,
            tc.tile_pool(name="acts", bufs=4) as actp,
            tc.tile_pool(name="outs", bufs=2) as outsp,
            tc.tile_pool(name="psb", bufs=1, space="PSUM") as psb,
        ):
            wtile = constp.tile([P, (3 + NCHUNK) * P], f32r)
            nc.sync.dma_start(wtile, wst[:, :])
            btile = constp.tile([P, 4], f32)
            nc.sync.dma_start(btile, bias[:, :])

            # Input DMA groups: first few single-sc loads so the pipeline
            # starts after ~0.5 MB instead of a full multi-sc transfer, then
            # steady-state groups of `gin` super-chunks.
            groups = [1] * min(4, nsc)
            while sum(groups) < nsc:
                groups.append(min(gin, nsc - sum(groups)))
            group_of = []
            for gidx, g in enumerate(groups):
                group_of += [(gidx, len(group_of), g)] * g
            group_starts = {}
            for s_, (gidx, gbase, g) in enumerate(group_of):
                group_starts.setdefault(gidx, (s_, g))

            xbig = None
            xbase = 0
            ob = None
            sc0_gate = None       # sc0's relu2 op, used to stagger stream B
            for s in range(nsc):
                gidx, gbase, gwidth = group_of[s]
                if s == gbase:
                    xbig = datap.tile([P, gwidth * SCW], f32r, tag="x")
                    xbase = gbase
                    nc.sync.dma_start(
                        xbig, fm[:, gbase * SCW:(gbase + gwidth) * SCW]
                    )
                si = s - xbase
                h = xbig[:, si * SCW:(si + 1) * SCW]

                # Four independent sc streams (s%4), each owning one
                # 2-bank PSUM slot: within a stream, relu(l) must complete
                # before mm(l+1) anyway, so one slot costs nothing, while
                # the streams interleave freely on every engine.
                for layer in range(3):
                    ps = psb.tile([P, SCW], f32, tag=f"ps{s % 4}")
                    wsl = wtile[:, layer * P:(layer + 1) * P]
                    for cc in range(NCHUNK):
                        mm = nc.tensor.matmul(
                            ps[:, cc * CH:(cc + 1) * CH],
                            wsl,
                            h[:, cc * CH:(cc + 1) * CH],
                            start=True,
                            stop=True,
                        )
                        NAME_INFO[mm.ins.name] = (s, f"mm{layer}.{cc}")
                        if stagger and s == 1 and layer == 0 and cc == 0 \
                                and sc0_gate is not None:
                            add_dep_helper(sc0_gate, mm.ins,
                                           reason="stagger stream B")
                    hn = actp.tile([P, SCW], f32r, tag=f"h{layer}")
                    bcol = btile[:, layer:layer + 1]
                    # Whole-crossing relu on one engine, alternating by
                    # (s + layer): each crossing is a single large op (best
                    # per-op amortization) and the two engines' dependency
                    # chains stay decoupled across layers.
                    if (s + layer) % 2 == 0:
                        xop = nc.scalar.activation(
                            hn[:, :], ps[:, :], Act.Relu,
                            bias=bcol, scale=1.0,
                        )
                        NAME_INFO[xop.ins.name] = (s, f"reluA{layer}")
                    else:
                        xop = nc.vector.tensor_scalar(
                            hn[:, :], ps[:, :],
                            bcol, 0.0, Alu.add, Alu.max,
                        )
                        NAME_INFO[xop.ins.name] = (s, f"reluD{layer}")
                    if s == 0 and layer == 1:
                        sc0_gate = xop.ins
                    h = hn

                # Layer 4: chunk c's [128,128] weight has wl only in columns
                # 4c+m (m<4); accumulating the 3 chunk matmuls into one bank
                # leaves out[4c+m, n] = wl @ (block m of chunk c) on the
                # contiguous partitions 0..11.
                go, so = divmod(s, gout)
                if so == 0:
                    ob = outsp.tile([OROWS, gout * CH], f32, tag="ob")
                ps4 = psb.tile([P, CH], f32, tag=f"ps{s % 4}")
                for cc in range(NCHUNK):
                    mm4 = nc.tensor.matmul(
                        ps4[:, :],
                        wtile[:, (3 + cc) * P:(4 + cc) * P],
                        h[:, cc * CH:(cc + 1) * CH],
                        start=(cc == 0),
                        stop=(cc == NCHUNK - 1),
                    )
                    NAME_INFO[mm4.ins.name] = (s, f"mm4.{cc}")
                blcol = btile[:OROWS, 3:4]
                if s % 2 == 0 and s % 8 != 0:
                    fin = nc.vector.tensor_scalar(
                        ob[:, so * CH:(so + 1) * CH], ps4[:OROWS, :],
                        blcol, None, Alu.add,
                    )
                else:
                    fin = nc.scalar.activation(
                        ob[:, so * CH:(so + 1) * CH], ps4[:OROWS, :],
                        Act.Identity, bias=blcol, scale=1.0,
                    )
                NAME_INFO[fin.ins.name] = (s, "final")
                # Store in two waves: the first ~2/3 of the output ships
                # while compute continues, so only the last third's DMA sits
                # in the drain tail.
                if gout == nsc:
                    wsplit = max(1, 8 * nsc // 9)
                    waves = {wsplit - 1: (0, wsplit), nsc - 1: (wsplit, nsc)}
                    if s in waves:
                        a, b = waves[s]
                        for cc in range(NCHUNK):
                            nc.sync.dma_start(
                                out_r[:, 0, a:b, cc, :],
                                ob[RG * cc:RG * cc + RG, a * CH:b * CH].rearrange(
                                    "m (g n) -> m g n", n=CH
                                ),
                            )
                elif so == gout - 1:
                    for cc in range(NCHUNK):
                        nc.sync.dma_start(
                            out_r[:, go, :, cc, :],
                            ob[RG * cc:RG * cc + RG, :].rearrange(
                                "m (g n) -> m g n", n=CH
                            ),
                        )

    # Walrus codegen cannot reliably attach semaphore waits to self-loading
    # matmuls; hoist every matmul's waits onto a PE nop inserted just before
    # it (sequencer-side wait, same semantics).
    for blk in nc.main_func.blocks:
        insts = blk.instructions
        idx = 0
        while idx < len(insts):
            inst = insts[idx]
            if isinstance(inst, mybir.InstMatmult):
                si = inst.sync_info
                if si is not None and len(si.on_wait) > 0:
                    nop = mybir.InstNoOp(
                        name=nc.get_next_instruction_name(), ins=[], outs=[]
                    )
                    nop.engine = inst.engine
                    nop.bass_nofuse = True
                    nop.sync_info = mybir.SyncInfo(on_wait=si.on_wait, on_update=[])
                    si.on_wait = []
                    nc.register_instruction(nop)
                    insts.insert(idx, nop)
                    idx += 1
            idx += 1

    for blk in nc.main_func.blocks:
        for inst in blk.instructions:
            if isinstance(inst, mybir.InstMatmult):
                si = inst.sync_info
                assert si is None or len(si.on_wait) == 0, inst.name

    nc.compile()
    return nc


def _blockdiag4(wT):
    """[32, 32] -> [128, 128] block-diagonal with 4 copies."""
    out = np.zeros((P, P), dtype=np.float32)
    for b in range(RG):
        out[32 * b:32 * b + 32, 32 * b:32 * b + 32] = wT
    return out


def _prep_host_inputs(z, w1, b1, w2, b2, w3, b3, wl, bl):
    """Fold z into the layer-1 bias and build the device weight layouts."""
    f32 = np.float32
    b1e = (b1 + w1[:, C:] @ z[0]).astype(f32)          # [32]

    w4 = np.zeros((P, NCHUNK * P), dtype=f32)
    for cc in range(NCHUNK):
        for m in range(RG):
            w4[32 * m:32 * m + 32, cc * P + RG * cc + m] = wl[0, :]

    wst = np.concatenate(
        [
            _blockdiag4(w1[:, :C].T),
            _blockdiag4(w2.T),
            _blockdiag4(w3.T),
            w4,
        ],
        axis=1,
    ).astype(f32)                                       # [128, 768]

    bias = np.zeros((P, 4), dtype=f32)
    bias[:, 0] = np.tile(b1e, RG)
    bias[:, 1] = np.tile(b2.astype(f32), RG)
    bias[:, 2] = np.tile(b3.astype(f32), RG)
    bias[:, 3] = f32(bl[0])
    return wst, bias


def _restripe(shard):
    """[32, npix] channel-major shard -> [128, npix/4] (block, channel) rows."""
    npix = shard.shape[1]
    return np.ascontiguousarray(
        shard.reshape(C, RG, npix // RG).transpose(1, 0, 2).reshape(P, npix // RG)
    )


_NC_CACHE = {}
NAME_INFO = {}   # instruction name -> (sc, stage) for profiling


def _run(feature_map, z, w1, b1, w2, b2, w3, b3, wl, bl, **spmd_kwargs):
    from concourse.bass_utils import run_bass_kernel_spmd

    feature_map = np.asarray(feature_map, dtype=np.float32)
    z = np.asarray(z, dtype=np.float32)
    w1, b1 = np.asarray(w1, np.float32), np.asarray(b1, np.float32)
    w2, b2 = np.asarray(w2, np.float32), np.asarray(b2, np.float32)
    w3, b3 = np.asarray(w3, np.float32), np.asarray(b3, np.float32)
    wl, bl = np.asarray(wl, np.float32), np.asarray(bl, np.float32)

    wst, bias = _prep_host_inputs(z, w1, b1, w2, b2, w3, b3, wl, bl)

    fm_flat = feature_map.reshape(C, VOL)
    in_maps = []
    for k in range(NCORES):
        shard = _restripe(fm_flat[:, k * NPIX:(k + 1) * NPIX])
        in_maps.append({"fm": shard, "wst": wst, "bias": bias})

    if "nc" not in _NC_CACHE:
        _NC_CACHE["nc"] = _build_nc()
    nc = _NC_CACHE["nc"]

    res = run_bass_kernel_spmd(nc, in_maps, core_ids=list(range(NCORES)), **spmd_kwargs)
    out = np.empty((VOL,), dtype=np.float32)
    for k in range(NCORES):
        out[k * NPIX:(k + 1) * NPIX] = res.results[k]["out"]
    return out.reshape(1, 1, 96, 96, 96), res


def kernel(feature_map, z, w1, b1, w2, b2, w3, b3, wl, bl):
    out, _ = _run(feature_map, z, w1, b1, w2, b2, w3, b3, wl, bl)
    return out



# revision 44
# speedup vs baseline: 1.1762x; 1.1762x over previous
"""Trainium2 Bass kernel for nn_FComb_79319456023150 (dense_cnn).

Per-pixel MLP over a 96^3 volume: four 1x1x1 convs (38->32->32->32->1 channels
with relu between). z is batch-constant, so w1[:, 32:38] @ z folds into the
layer-1 bias and every layer becomes a K=32 channel GEMM.

Sharding: spatial (outermost X axis) across 8 cores, 110592 pixels each.
Weights/biases replicated.

Device layout per core: the host restripes each shard to [128, 27648] = 4
pixel-blocks x 32 channels on partitions, pixels on the free dim, in bf16
(rel err ~5e-3, well under the 2e-2 gate; halves DMA vs fp32). Each layer is
computed with a BLOCK-DIAGONAL [128, 128] weight (4 copies of W^T on the
diagonal), so one full-array bf16 matmul per 512-col super-chunk applies the
32x32 GEMM to all 4 pixel blocks at once (1 col/cycle).

Pipeline: SEVEN super-chunk streams, one PSUM bank each; bank 7 is a
dedicated L4 accumulator. Relu+bias rides the mandatory PSUM->SBUF crossing,
which only Act and DVE may perform (GPSIMD cannot access PSUM), and their
combined throughput barely covers the demand - so crossings are COHORT ops
spanning adjacent streams' banks (contiguous PSUM): [128, 1024] over a
stream pair amortizes the per-op init (Act 185ns / DVE 125ns) that a
512-wide op can't afford. The 4th (mm4) row of each group is crossing-free
slack that drains the transient backlog.

L4: each sc's matmul accumulates into bank 7 at rows 4j+m (j = sc index
within a 28-sc half), so HALF THE KERNEL's output evacuates with ONE
[112, 512] op (engines charge free size only) and ships with one DMA.
"""

import sys

import numpy as np

if "/opt/trn_rl_repo" not in sys.path:
    sys.path.insert(0, "/opt/trn_rl_repo")

C = 32          # channels per layer
P = 128         # SBUF/PSUM partitions
RG = 4          # pixel blocks stacked on the partition dim (128/32)
CH = 512        # super-chunk width = one PSUM bank of fp32
VOL = 96 * 96 * 96                   # full volume
NCORES = 8
NPIX = VOL // NCORES                 # 110592 pixels per core
FREE = NPIX // RG                    # 27648 free-dim columns per core
NSC = FREE // CH                     # 54 super-chunks per core
NS = 7                               # parallel sc streams (PSUM banks 0-6)
L4SPAN = 28                          # scs accumulated per L4-bank fill
assert FREE % CH == 0


# Crossing cohorts: layer-l results of streams [0,1], [2,3], [4,5] evacuate
# as [128,1024] ops; stream 6 as [128,512]. "pat" assigns engines per
# (group-parity, layer row) to the 4 cohort ops; Act is faster per column
# (1.2 vs 0.96 GHz) so it leans on the wide ops.
DEFAULT_CFG = {
    "pat": [
        [("act", "dve", "act", "dve"), ("dve", "act", "dve", "act"),
         ("act", "dve", "act", "dve")],
    ],
    "fin": "act",
    "g0split": [3, 4],
    "cohorts": [(0, 2), (2, 2), (4, 2), (6, 1)],
}


def _build_nc(npix=NPIX, cfg=None):
    import concourse.mybir as mybir
    from concourse import bacc
    from concourse.tile import TileContext

    f32 = mybir.dt.float32
    bf16 = mybir.dt.bfloat16
    Alu = mybir.AluOpType
    Act = mybir.ActivationFunctionType

    if cfg is None:
        cfg = DEFAULT_CFG
    pat = cfg["pat"]
    fin_cfg = cfg.get("fin", "act")
    cohorts = cfg.get("cohorts", [(0, 2), (2, 2), (4, 2), (6, 1)])

    free = npix // RG
    nsc = free // CH
    assert free % CH == 0 and nsc >= 1
    l4span = min(L4SPAN, nsc)

    nc = bacc.Bacc()
    fm = nc.dram_tensor("fm", [P, free], bf16, kind="ExternalInput")
    wst = nc.dram_tensor(
        "wst", [P, (3 + l4span) * P], bf16, kind="ExternalInput")
    bias = nc.dram_tensor("bias", [P, 4], f32, kind="ExternalInput")
    out = nc.dram_tensor("out", [npix], f32, kind="ExternalOutput")

    # out[m*free + s*CH + n] viewed [m, s, n] for per-half stores
    out_r = out.rearrange("(m s n) -> m s n", m=RG, s=nsc, n=CH)

    sgroups = [list(range(g, min(g + NS, nsc)))
               for g in range(0, nsc, NS)]

    with TileContext(nc) as tc:
        with (
            tc.tile_pool(name="const", bufs=1) as constp,
            tc.tile_pool(name="data", bufs=cfg.get("xbufs", 4)) as datap,
            tc.tile_pool(name="acts", bufs=cfg.get("hbufs", 2)) as actp,
            tc.tile_pool(name="outs", bufs=2) as outsp,
            tc.tile_pool(name="psb", bufs=1, space="PSUM") as psb,
        ):
            # Startup critical path: the first matmul needs only the three
            # layer weights + the first data chunk; bias and the L4 weights
            # ride behind the first data batches.
            wtile = constp.tile([P, (3 + l4span) * P], bf16)
            btile = constp.tile([P, 4], f32)
            nc.sync.dma_start(wtile[:, :3 * P], wst[:, :3 * P])

            psum = psb.tile([P, 8 * CH], f32)
            l4ps = psum[:, NS * CH:(NS + 1) * CH]

            def xop_on(eng, out_ap, in_ap, bcol, relu):
                if eng == "act":
                    return nc.scalar.activation(
                        out_ap, in_ap, Act.Relu if relu else Act.Identity,
                        bias=bcol, scale=1.0,
                    )
                e = nc.vector if eng == "dve" else nc.gpsimd
                if relu:
                    return e.tensor_scalar(out_ap, in_ap, bcol, 0.0,
                                           Alu.add, Alu.max)
                return e.tensor_scalar(out_ap, in_ap, bcol, None, Alu.add)

            hcur = {}
            hl4 = {}
            pending_mm4 = []
            for gi, scs in enumerate(sgroups):
                # input DMA: the first group loads in pieces so early
                # streams start while later ones transfer; later groups as
                # one batched DMA each.
                if scs[0] == 0:
                    base = 0
                    for bi, blen in enumerate(cfg.get("g0split", [4, 3])):
                        xt = datap.tile([P, blen * CH], bf16, tag="x")
                        nc.sync.dma_start(
                            xt, fm[:, base * CH:(base + blen) * CH])
                        for i in range(blen):
                            hcur[base + i] = xt[:, i * CH:(i + 1) * CH]
                        base += blen
                        if bi == 0:
                            nc.sync.dma_start(btile, bias[:, :])
                        if bi == 1 or (bi == 0 and base == len(scs)):
                            nc.sync.dma_start(
                                wtile[:, 3 * P:], wst[:, 3 * P:])
                    assert base == len(scs)
                else:
                    xt = datap.tile([P, len(scs) * CH], bf16, tag="x")
                    nc.sync.dma_start(
                        xt, fm[:, scs[0] * CH:(scs[0] + len(scs)) * CH])
                    for i, s in enumerate(scs):
                        hcur[s] = xt[:, i * CH:(i + 1) * CH]

                def emit_mm4(mm4_scs):
                    # layer 4: sc s accumulates into the dedicated bank at
                    # rows l4span*m + (s % l4span); each l4span-half
                    # evacuates with ONE [4*l4span, 512] op + 4 plain DMAs.
                    for s in mm4_scs:
                        jj = s % l4span
                        mm4 = nc.tensor.matmul(
                            l4ps, wtile[:, (3 + jj) * P:(4 + jj) * P],
                            hl4[s],
                            start=(jj == 0), stop=(jj == l4span - 1
                                                   or s == nsc - 1),
                        )
                        NAME_INFO[mm4.ins.name] = (s, "mm4")
                        if jj == l4span - 1 or s == nsc - 1:
                            hbase = s - jj
                            nrow = RG * l4span
                            ob = outsp.tile([RG * l4span, CH], f32,
                                            tag="ob")
                            fin = xop_on(fin_cfg, ob[:nrow, :],
                                         l4ps[:nrow, :],
                                         btile[:nrow, 3:4], relu=False)
                            NAME_INFO[fin.ins.name] = (
                                hbase, f"final.{fin_cfg}")
                            # ONE DMA per half: the SBUF side stays a
                            # plain single-partition-dim [4*(jj+1), 512]
                            # (2-partition-dim SBUF APs mis-lower in walrus
                            # DMA codegen); the permutation lives on the
                            # DRAM side as nested strides (j, m, n).
                            dmao = nc.sync.dma_start(
                                out_r[:, hbase:s + 1, :].rearrange(
                                    "m k n -> k m n"),
                                ob[:RG * (jj + 1), :],
                            )
                            NAME_INFO[dmao.ins.name] = (hbase, "dma_out")

                rowpat = pat[gi % len(pat)]
                for layer in range(3):
                    for j, s in enumerate(scs):
                        ps = psum[:, j * CH:(j + 1) * CH]
                        mm = nc.tensor.matmul(
                            ps, wtile[:, layer * P:(layer + 1) * P],
                            hcur[s], start=True, stop=True,
                        )
                        NAME_INFO[mm.ins.name] = (s, f"mm{layer}")
                    if layer == 1 and pending_mm4:
                        # previous group's L4 row rides here, so the
                        # engines' relu work stays contiguous across the
                        # group boundary instead of idling through two
                        # crossing-free PE rows.
                        emit_mm4(pending_mm4)
                        pending_mm4 = []
                    bcol = btile[:, layer:layer + 1]
                    for ci, (c0, clen) in enumerate(cohorts):
                        if c0 >= len(scs):
                            continue
                        cl = min(clen, len(scs) - c0)
                        ps = psum[:, c0 * CH:(c0 + cl) * CH]
                        hn = actp.tile([P, cl * CH], bf16,
                                       tag=f"h{layer}c{ci}")
                        eng = rowpat[layer][ci]
                        xop = xop_on(eng, hn[:, :], ps, bcol, relu=True)
                        NAME_INFO[xop.ins.name] = (
                            scs[c0], f"relu{layer}.c{ci}.{eng}")
                        for i in range(cl):
                            hcur[scs[c0 + i]] = hn[:, i * CH:(i + 1) * CH]

                for s in scs:
                    hl4[s] = hcur[s]
                pending_mm4 = pending_mm4 + list(scs)

            if pending_mm4:
                emit_mm4(pending_mm4)

    nc.compile()
    return nc


def _blockdiag4(wT):
    """[32, 32] -> [128, 128] block-diagonal with 4 copies."""
    out = np.zeros((P, P), dtype=np.float32)
    for b in range(RG):
        out[32 * b:32 * b + 32, 32 * b:32 * b + 32] = wT
    return out


def _prep_host_inputs(z, w1, b1, w2, b2, w3, b3, wl, bl):
    """Fold z into the layer-1 bias and build the device weight layouts."""
    import ml_dtypes

    f32 = np.float32
    b1e = (b1 + w1[:, C:] @ z[0]).astype(f32)          # [32]

    # w4 block j: L4-bank row 4j+m <- wl . (pixel-block m of half-member
    # j's sc) - j-major, matching the output DMA's (j, m, n) iteration.
    l4span = min(L4SPAN, NSC)
    w4 = np.zeros((P, l4span * P), dtype=f32)
    for j in range(l4span):
        for m in range(RG):
            w4[32 * m:32 * m + 32, j * P + RG * j + m] = wl[0, :]

    wst = np.concatenate(
        [
            _blockdiag4(w1[:, :C].T),
            _blockdiag4(w2.T),
            _blockdiag4(w3.T),
            w4,
        ],
        axis=1,
    ).astype(ml_dtypes.bfloat16)                        # [128, (3+28)*128]

    bias = np.zeros((P, 4), dtype=f32)
    bias[:, 0] = np.tile(b1e, RG)
    bias[:, 1] = np.tile(b2.astype(f32), RG)
    bias[:, 2] = np.tile(b3.astype(f32), RG)
    bias[:, 3] = f32(bl[0])
    return wst, bias


def _restripe(shard):
    """[32, npix] channel-major shard -> [128, npix/4] (block, channel) rows."""
    npix = shard.shape[1]
    return np.ascontiguousarray(
        shard.reshape(C, RG, npix // RG).transpose(1, 0, 2).reshape(P, npix // RG)
    )


_NC_CACHE = {}
NAME_INFO = {}   # instruction name -> (sc, stage) for profiling


def _run(feature_map, z, w1, b1, w2, b2, w3, b3, wl, bl, **spmd_kwargs):
    import ml_dtypes
    from concourse.bass_utils import run_bass_kernel_spmd

    feature_map = np.asarray(feature_map, dtype=np.float32)
    z = np.asarray(z, dtype=np.float32)
    w1, b1 = np.asarray(w1, np.float32), np.asarray(b1, np.float32)
    w2, b2 = np.asarray(w2, np.float32), np.asarray(b2, np.float32)
    w3, b3 = np.asarray(w3, np.float32), np.asarray(b3, np.float32)
    wl, bl = np.asarray(wl, np.float32), np.asarray(bl, np.float32)

    wst, bias = _prep_host_inputs(z, w1, b1, w2, b2, w3, b3, wl, bl)

    fm_flat = feature_map.reshape(C, VOL)
    in_maps = []
    for k in range(NCORES):
        shard = _restripe(fm_flat[:, k * NPIX:(k + 1) * NPIX]).astype(
            ml_dtypes.bfloat16
        )
        in_maps.append({"fm": shard, "wst": wst, "bias": bias})

    if "nc" not in _NC_CACHE:
        _NC_CACHE["nc"] = _build_nc()
    nc = _NC_CACHE["nc"]

    res = run_bass_kernel_spmd(nc, in_maps, core_ids=list(range(NCORES)), **spmd_kwargs)
    out = np.empty((VOL,), dtype=np.float32)
    for k in range(NCORES):
        out[k * NPIX:(k + 1) * NPIX] = res.results[k]["out"]
    return out.reshape(1, 1, 96, 96, 96), res


def kernel(feature_map, z, w1, b1, w2, b2, w3, b3, wl, bl):
    out, _ = _run(feature_map, z, w1, b1, w2, b2, w3, b3, wl, bl)
    return out
